# revision 2
# baseline (speedup 1.0000x reference)
"""Batched Kalman filter update on 8 trn2 NeuronCores.

Strategy (pure data parallel over the track dim B=262144, 32768/core):
- Natural layout in SBUF: partition = batch-high, per-partition columns hold
  each element's [x(8) | z(4) | P(64) | 1] = 77 floats contiguously.
- TensorE bridge per 128-element block: transpose [128,77] -> [77,128]
  (entries-on-partitions), then one fp32 matmul with a host-baked weight
  matrix W1 [77,46] computes U=P.H^T (32), S=H.P.H^T+R upper (10), y=z-Hx (4)
  straight back in natural layout [128,46].
- VectorE solves the 4x4 SPD system via LDL^T per element with strided /
  broadcast access patterns (all ops [128, nq, w], full 128-lane utilization):
    S = L D L^T;  W = U L^-T;  v = L^-1 y
    x_new = x + sum_j w_j * v_j / d_j
    P_new = P - sum_j (w_j/sqrt(d_j)) (w_j/sqrt(d_j))^T   (upper + mirror)
  Algebraically identical to K = U S^-1 (Joseph form equals this exactly).
"""

import numpy as np

import concourse.bacc as bacc
import concourse.tile as tile
from concourse import mybir

NCORES = 8
B = 262144
BC = B // NCORES          # 32768 per core
P128 = 128
N = BC // P128            # 256 elements per partition
Q = 2                     # sub-chunks for DMA/compute overlap
NQ = N // Q               # 128 columns per sub-chunk

F32 = mybir.dt.float32
MUL = mybir.AluOpType.mult
SUB = mybir.AluOpType.subtract
ADD = mybir.AluOpType.add

# upper-triangle index order for S (4x4): (m,l) m<=l
SUP = [(0, 0), (0, 1), (0, 2), (0, 3), (1, 1), (1, 2), (1, 3), (2, 2), (2, 3), (3, 3)]


def _build_w1(H: np.ndarray, R: np.ndarray) -> np.ndarray:
    """W1 [77, 46]: rows = [x(0:8) | z(8:12) | P(12:76) | ones(76)],
    cols = [U(i*4+m) 0:32 | S upper 32:42 | y 42:46]."""
    W1 = np.zeros((77, 46), dtype=np.float32)
    # U[i,m] = sum_k P[i,k] H[m,k]
    for i in range(8):
        for m in range(4):
            for k in range(8):
                W1[12 + i * 8 + k, i * 4 + m] = H[m, k]
    # S[m,l] = sum_{i,k} H[m,i] H[l,k] P[i,k] + R[m,l]
    for idx, (m, l) in enumerate(SUP):
        for i in range(8):
            for k in range(8):
                W1[12 + i * 8 + k, 32 + idx] += H[m, i] * H[l, k]
        W1[76, 32 + idx] = R[m, l]
    # y_m = z_m - sum_k H[m,k] x_k
    for m in range(4):
        W1[8 + m, 42 + m] = 1.0
        for k in range(8):
            W1[k, 42 + m] = -H[m, k]
    return W1


def _build_program():
    nc = bacc.Bacc("TRN2", target_bir_lowering=False, debug=False,
                   num_devices=NCORES)
    xd = nc.dram_tensor("xd", [BC, 8], F32, kind="ExternalInput")
    zd = nc.dram_tensor("zd", [BC, 4], F32, kind="ExternalInput")
    Pd = nc.dram_tensor("Pd", [BC, 64], F32, kind="ExternalInput")
    w1d = nc.dram_tensor("w1d", [77, 46], F32, kind="ExternalInput")
    idd = nc.dram_tensor("idd", [128, 128], F32, kind="ExternalInput")
    outd = nc.dram_tensor("outd", [BC, 72], F32, kind="ExternalOutput")

    xv = xd.ap().rearrange("(p f) c -> p f c", p=P128)    # [128, N, 8]
    zv = zd.ap().rearrange("(p f) c -> p f c", p=P128)
    Pv = Pd.ap().rearrange("(p f) c -> p f c", p=P128)
    ov = outd.ap().rearrange("(p f) c -> p f c", p=P128)  # [128, N, 72]

    with tile.TileContext(nc) as tc:
        with (
            tc.tile_pool(name="consts", bufs=1) as consts,
            tc.tile_pool(name="xpz", bufs=2) as xpz_pool,
            tc.tile_pool(name="ut", bufs=2) as ut_pool,
            tc.tile_pool(name="sc", bufs=2) as sc_pool,
            tc.tile_pool(name="xvrt", bufs=3) as xv_pool,
            tc.tile_pool(name="tps", bufs=3, space="PSUM") as tp_ps,
            tc.tile_pool(name="usps", bufs=3, space="PSUM") as us_ps,
        ):
            w1s = consts.tile([77, 46], F32)
            nc.sync.dma_start(out=w1s, in_=w1d.ap())
            ids = consts.tile([128, 128], F32)
            nc.sync.dma_start(out=ids, in_=idd.ap())

            for q in range(Q):
                f0 = q * NQ
                FS = slice(f0, f0 + NQ)

                XPZ = xpz_pool.tile([P128, NQ, 77], F32, tag="xpz")
                UT = ut_pool.tile([P128, NQ, 46], F32, tag="ut")
                SC = sc_pool.tile([P128, NQ, 26], F32, tag="sc")

                nc.sync.dma_start(out=XPZ[:, :, 0:8], in_=xv[:, FS, :])
                nc.sync.dma_start(out=XPZ[:, :, 8:12], in_=zv[:, FS, :])
                nc.sync.dma_start(out=XPZ[:, :, 12:76], in_=Pv[:, FS, :])
                nc.vector.memset(XPZ[:, :, 76:77], 1.0)

                # ---- TensorE bridge: transpose + linear pass, 2 blocks/copy ----
                for f in range(0, NQ, 2):
                    tp = tp_ps.tile([77, 256], F32, tag="tp")
                    nc.tensor.transpose(tp[:, 0:128], XPZ[:, f, :], ids)
                    nc.tensor.transpose(tp[:, 128:256], XPZ[:, f + 1, :], ids)
                    xvert = xv_pool.tile([77, 256], F32, tag="xvert")
                    nc.scalar.copy(xvert, tp)
                    us = us_ps.tile([128, 92], F32, tag="us")
                    nc.tensor.matmul(us[:, 0:46], xvert[:, 0:128], w1s)
                    nc.tensor.matmul(us[:, 46:92], xvert[:, 128:256], w1s)
                    nc.scalar.copy(UT[:, f:f + 2, :],
                                   us.rearrange("p (f c) -> p f c", f=2))

                # ---- helpers -------------------------------------------
                def U(c0, w=1):
                    return UT[:, :, c0:c0 + w]

                def S(c0, w=1):
                    return SC[:, :, c0:c0 + w]

                def bc(ap, w):
                    return ap.broadcast_to([P128, NQ, w])

                tmp = SC[:, :, 18:26]       # 8-wide scratch

                def T(out, a, b, op):
                    nc.vector.tensor_tensor(out=out, in0=a, in1=b, op=op)

                # ---- LDL of S (in place in UT cols 32..41) -------------
                # cols: s00=32 s01=33 s02=34 s03=35 s11=36 s12=37 s13=38
                #       s22=39 s23=40 s33=41 ; y/v = 42..45
                nc.vector.reciprocal(S(6), U(32))                # rec0
                T(S(0, 3), U(33, 3), bc(S(6), 3), MUL)           # l10,l20,l30
                T(tmp[:, :, 0:3], bc(S(0), 3), U(33, 3), MUL)
                T(U(36, 3), U(36, 3), tmp[:, :, 0:3], SUB)       # s11,s12,s13
                T(tmp[:, :, 0:2], bc(S(1), 2), U(34, 2), MUL)
                T(U(39, 2), U(39, 2), tmp[:, :, 0:2], SUB)       # s22,s23
                T(tmp[:, :, 0:1], S(2), U(35), MUL)
                T(U(41), U(41), tmp[:, :, 0:1], SUB)             # s33
                nc.vector.reciprocal(S(7), U(36))                # rec1
                T(S(3, 2), U(37, 2), bc(S(7), 2), MUL)           # l21,l31
                T(tmp[:, :, 0:2], bc(S(3), 2), U(37, 2), MUL)
                T(U(39, 2), U(39, 2), tmp[:, :, 0:2], SUB)
                T(tmp[:, :, 0:1], S(4), U(38), MUL)
                T(U(41), U(41), tmp[:, :, 0:1], SUB)
                nc.vector.reciprocal(S(8), U(39))                # rec2
                T(S(5), U(40), S(8), MUL)                        # l32
                T(tmp[:, :, 0:1], S(5), U(40), MUL)
                T(U(41), U(41), tmp[:, :, 0:1], SUB)
                nc.vector.reciprocal(S(9), U(41))                # rec3
                nc.scalar.activation(S(10, 4), S(6, 4),
                                     mybir.ActivationFunctionType.Sqrt)

                # ---- v = L^-1 y (in place in UT 42..45), atil ----------
                T(tmp[:, :, 0:3], S(0, 3), bc(U(42), 3), MUL)
                T(U(43, 3), U(43, 3), tmp[:, :, 0:3], SUB)
                T(tmp[:, :, 0:2], S(3, 2), bc(U(43), 2), MUL)
                T(U(44, 2), U(44, 2), tmp[:, :, 0:2], SUB)
                T(tmp[:, :, 0:1], S(5), U(44), MUL)
                T(U(45), U(45), tmp[:, :, 0:1], SUB)
                T(S(14, 4), U(42, 4), S(10, 4), MUL)             # atil = v*sqrtrec

                # ---- W solve in place over U cols ----------------------
                Uv = UT[:, :, 0:32].rearrange("p f (i m) -> p f i m", m=4)

                def um(m):
                    return Uv[:, :, :, m]                        # [128,NQ,8] stride 4

                for (m, j, lc) in ((1, 0, 0), (2, 0, 1), (2, 1, 3),
                                   (3, 0, 2), (3, 1, 4), (3, 2, 5)):
                    # u_m -= l(m,j) * w_j
                    T(tmp, um(j), bc(S(lc), 8), MUL)
                    T(um(m), um(m), tmp, SUB)
                for j in range(4):                                # scale: wtil
                    T(um(j), um(j), bc(S(10 + j), 8), MUL)

                # ---- x update ------------------------------------------
                X = XPZ[:, :, 0:8]
                for j in range(4):
                    T(tmp, um(j), bc(S(14 + j), 8), MUL)
                    T(X, X, tmp, ADD)

                # ---- P update (upper), then mirror ---------------------
                for j in range(4):
                    for i in range(8):
                        w = 8 - i
                        lhs = bc(UT[:, :, i * 4 + j:i * 4 + j + 1], w)
                        rhs = Uv[:, :, i:8, j]
                        T(tmp[:, :, 0:w], lhs, rhs, MUL)
                        prun = XPZ[:, :, 12 + i * 8 + i: 12 + i * 8 + 8]
                        T(prun, prun, tmp[:, :, 0:w], SUB)

                P2 = XPZ[:, :, 12:76].rearrange("p f (i k) -> p f i k", i=8)
                for i in range(1, 8):
                    nc.scalar.copy(P2[:, :, i, 0:i], P2[:, :, 0:i, i])

                # ---- DMA out -------------------------------------------
                nc.sync.dma_start(out=ov[:, FS, 0:8], in_=XPZ[:, :, 0:8])
                nc.sync.dma_start(out=ov[:, FS, 8:72], in_=XPZ[:, :, 12:76])

    nc.compile()
    return nc


_prog_cache = {}


def kernel(x: np.ndarray, z: np.ndarray, P: np.ndarray,
           H: np.ndarray, R: np.ndarray) -> np.ndarray:
    from concourse.bass_utils import run_bass_kernel_spmd

    x = np.ascontiguousarray(x, dtype=np.float32).reshape(B, 8)
    z = np.ascontiguousarray(z, dtype=np.float32).reshape(B, 4)
    P = np.ascontiguousarray(P, dtype=np.float32).reshape(B, 64)
    W1 = _build_w1(np.asarray(H, np.float32), np.asarray(R, np.float32))
    ident = np.eye(128, dtype=np.float32)

    if "nc" not in _prog_cache:
        _prog_cache["nc"] = _build_program()
    nc = _prog_cache["nc"]

    in_maps = []
    for c in range(NCORES):
        s = slice(c * BC, (c + 1) * BC)
        in_maps.append({"xd": x[s], "zd": z[s], "Pd": P[s],
                        "w1d": W1, "idd": ident})
    res = run_bass_kernel_spmd(nc, in_maps, core_ids=list(range(NCORES)))
    out = np.concatenate([r["outd"].reshape(BC, 9, 8) for r in res.results],
                         axis=0)
    return out


# revision 3
# speedup vs baseline: 1.3667x; 1.3667x over previous
"""Batched Kalman filter update on 8 trn2 NeuronCores.

Strategy (pure data parallel over the track dim B=262144, 32768/core):
- Natural layout in SBUF: partition = batch-high, per-partition columns hold
  each element's [x(8) | z(4) | P(64) | 1] = 77 floats contiguously.
- TensorE bridge per 128-element block: transpose [128,77] -> [77,128]
  (entries-on-partitions), then one fp32 matmul with a host-baked weight
  matrix W1 [77,46] computes U=P.H^T (32), S=H.P.H^T+R upper (10), y=z-Hx (4)
  straight back in natural layout [128,46].
- VectorE solves the 4x4 SPD system via LDL^T per element with strided /
  broadcast access patterns (all ops [128, nq, w], full 128-lane utilization):
    S = L D L^T;  W = U L^-T;  v = L^-1 y
    x_new = x + sum_j w_j * v_j / d_j
    P_new = P - sum_j (w_j/sqrt(d_j)) (w_j/sqrt(d_j))^T   (upper + mirror)
  Algebraically identical to K = U S^-1 (Joseph form equals this exactly).
"""

import numpy as np

import concourse.bacc as bacc
import concourse.tile as tile
from concourse import mybir

NCORES = 8
B = 262144
BC = B // NCORES          # 32768 per core
P128 = 128
N = BC // P128            # 256 elements per partition
Q = 2                     # sub-chunks for DMA/compute overlap
NQ = N // Q               # 128 columns per sub-chunk

F32 = mybir.dt.float32
MUL = mybir.AluOpType.mult
SUB = mybir.AluOpType.subtract
ADD = mybir.AluOpType.add

# upper-triangle index order for S (4x4): (m,l) m<=l
SUP = [(0, 0), (0, 1), (0, 2), (0, 3), (1, 1), (1, 2), (1, 3), (2, 2), (2, 3), (3, 3)]


def _build_w1(H: np.ndarray, R: np.ndarray) -> np.ndarray:
    """W1 [77, 46]: rows = [x(0:8) | z(8:12) | P(12:76) | ones(76)],
    cols = [U(i*4+m) 0:32 | S upper 32:42 | y 42:46]."""
    W1 = np.zeros((77, 46), dtype=np.float32)
    # U[i,m] = sum_k P[i,k] H[m,k]
    for i in range(8):
        for m in range(4):
            for k in range(8):
                W1[12 + i * 8 + k, i * 4 + m] = H[m, k]
    # S[m,l] = sum_{i,k} H[m,i] H[l,k] P[i,k] + R[m,l]
    for idx, (m, l) in enumerate(SUP):
        for i in range(8):
            for k in range(8):
                W1[12 + i * 8 + k, 32 + idx] += H[m, i] * H[l, k]
        W1[76, 32 + idx] = R[m, l]
    # y_m = z_m - sum_k H[m,k] x_k
    for m in range(4):
        W1[8 + m, 42 + m] = 1.0
        for k in range(8):
            W1[k, 42 + m] = -H[m, k]
    return W1


def _build_program():
    nc = bacc.Bacc("TRN2", target_bir_lowering=False, debug=False,
                   num_devices=NCORES)
    xd = nc.dram_tensor("xd", [BC, 8], F32, kind="ExternalInput")
    zd = nc.dram_tensor("zd", [BC, 4], F32, kind="ExternalInput")
    Pd = nc.dram_tensor("Pd", [BC, 64], F32, kind="ExternalInput")
    w1d = nc.dram_tensor("w1d", [77, 46], F32, kind="ExternalInput")
    idd = nc.dram_tensor("idd", [128, 128], F32, kind="ExternalInput")
    outd = nc.dram_tensor("outd", [BC, 72], F32, kind="ExternalOutput")

    xv = xd.ap().rearrange("(p f) c -> p f c", p=P128)    # [128, N, 8]
    zv = zd.ap().rearrange("(p f) c -> p f c", p=P128)
    Pv = Pd.ap().rearrange("(p f) c -> p f c", p=P128)
    ov = outd.ap().rearrange("(p f) c -> p f c", p=P128)  # [128, N, 72]

    with tile.TileContext(nc) as tc:
        with (
            tc.tile_pool(name="consts", bufs=1) as consts,
            tc.tile_pool(name="xpz", bufs=2) as xpz_pool,
            tc.tile_pool(name="ut", bufs=2) as ut_pool,
            tc.tile_pool(name="sc", bufs=2) as sc_pool,
            tc.tile_pool(name="xvrt", bufs=3) as xv_pool,
            tc.tile_pool(name="tps", bufs=3, space="PSUM") as tp_ps,
            tc.tile_pool(name="usps", bufs=3, space="PSUM") as us_ps,
        ):
            w1s = consts.tile([77, 46], F32)
            nc.sync.dma_start(out=w1s, in_=w1d.ap())
            ids = consts.tile([128, 128], F32)
            nc.sync.dma_start(out=ids, in_=idd.ap())

            for q in range(Q):
                f0 = q * NQ
                FS = slice(f0, f0 + NQ)

                XPZ = xpz_pool.tile([P128, NQ, 77], F32, tag="xpz")
                UT = ut_pool.tile([P128, NQ, 46], F32, tag="ut")
                SC = sc_pool.tile([P128, NQ, 26], F32, tag="sc")

                nc.sync.dma_start(out=XPZ[:, :, 0:8], in_=xv[:, FS, :])
                nc.sync.dma_start(out=XPZ[:, :, 8:12], in_=zv[:, FS, :])
                nc.sync.dma_start(out=XPZ[:, :, 12:76], in_=Pv[:, FS, :])
                nc.vector.memset(XPZ[:, :, 76:77], 1.0)

                # ---- TensorE bridge: transpose + linear pass, 4 blocks/copy ----
                for f in range(0, NQ, 4):
                    tp = tp_ps.tile([77, 512], F32, tag="tp")
                    for g in range(4):
                        nc.tensor.transpose(tp[:, g * 128:(g + 1) * 128],
                                            XPZ[:, f + g, :], ids)
                    xvert = xv_pool.tile([77, 512], F32, tag="xvert")
                    nc.scalar.copy(xvert, tp)
                    us = us_ps.tile([128, 184], F32, tag="us")
                    for g in range(4):
                        nc.tensor.matmul(us[:, g * 46:(g + 1) * 46],
                                         xvert[:, g * 128:(g + 1) * 128], w1s)
                    nc.scalar.copy(UT[:, f:f + 4, :],
                                   us.rearrange("p (f c) -> p f c", f=4))

                # ---- helpers -------------------------------------------
                def U(c0, w=1):
                    return UT[:, :, c0:c0 + w]

                def S(c0, w=1):
                    return SC[:, :, c0:c0 + w]

                def bc(ap, w):
                    return ap.broadcast_to([P128, NQ, w])

                tmp = SC[:, :, 18:26]       # 8-wide scratch

                def T(out, a, b, op):
                    nc.vector.tensor_tensor(out=out, in0=a, in1=b, op=op)

                # ---- LDL of S (in place in UT cols 32..41) -------------
                # cols: s00=32 s01=33 s02=34 s03=35 s11=36 s12=37 s13=38
                #       s22=39 s23=40 s33=41 ; y/v = 42..45
                nc.vector.reciprocal(S(6), U(32))                # rec0
                T(S(0, 3), U(33, 3), bc(S(6), 3), MUL)           # l10,l20,l30
                T(tmp[:, :, 0:3], bc(S(0), 3), U(33, 3), MUL)
                T(U(36, 3), U(36, 3), tmp[:, :, 0:3], SUB)       # s11,s12,s13
                T(tmp[:, :, 0:2], bc(S(1), 2), U(34, 2), MUL)
                T(U(39, 2), U(39, 2), tmp[:, :, 0:2], SUB)       # s22,s23
                T(tmp[:, :, 0:1], S(2), U(35), MUL)
                T(U(41), U(41), tmp[:, :, 0:1], SUB)             # s33
                nc.vector.reciprocal(S(7), U(36))                # rec1
                T(S(3, 2), U(37, 2), bc(S(7), 2), MUL)           # l21,l31
                T(tmp[:, :, 0:2], bc(S(3), 2), U(37, 2), MUL)
                T(U(39, 2), U(39, 2), tmp[:, :, 0:2], SUB)
                T(tmp[:, :, 0:1], S(4), U(38), MUL)
                T(U(41), U(41), tmp[:, :, 0:1], SUB)
                nc.vector.reciprocal(S(8), U(39))                # rec2
                T(S(5), U(40), S(8), MUL)                        # l32
                T(tmp[:, :, 0:1], S(5), U(40), MUL)
                T(U(41), U(41), tmp[:, :, 0:1], SUB)
                nc.vector.reciprocal(S(9), U(41))                # rec3
                nc.scalar.activation(S(10, 4), S(6, 4),
                                     mybir.ActivationFunctionType.Sqrt)

                # ---- v = L^-1 y (in place in UT 42..45), atil ----------
                T(tmp[:, :, 0:3], S(0, 3), bc(U(42), 3), MUL)
                T(U(43, 3), U(43, 3), tmp[:, :, 0:3], SUB)
                T(tmp[:, :, 0:2], S(3, 2), bc(U(43), 2), MUL)
                T(U(44, 2), U(44, 2), tmp[:, :, 0:2], SUB)
                T(tmp[:, :, 0:1], S(5), U(44), MUL)
                T(U(45), U(45), tmp[:, :, 0:1], SUB)
                T(S(14, 4), U(42, 4), S(10, 4), MUL)             # atil = v*sqrtrec

                # ---- W solve in place over U cols ----------------------
                Uv = UT[:, :, 0:32].rearrange("p f (i m) -> p f i m", m=4)

                def um(m):
                    return Uv[:, :, :, m]                        # [128,NQ,8] stride 4

                for (m, j, lc) in ((1, 0, 0), (2, 0, 1), (2, 1, 3),
                                   (3, 0, 2), (3, 1, 4), (3, 2, 5)):
                    # u_m -= l(m,j) * w_j
                    T(tmp, um(j), bc(S(lc), 8), MUL)
                    T(um(m), um(m), tmp, SUB)
                for j in range(4):                                # scale: wtil
                    T(um(j), um(j), bc(S(10 + j), 8), MUL)

                # ---- x update ------------------------------------------
                X = XPZ[:, :, 0:8]
                for j in range(4):
                    T(tmp, um(j), bc(S(14 + j), 8), MUL)
                    T(X, X, tmp, ADD)

                # ---- P update (upper), then mirror ---------------------
                for j in range(4):
                    for i in range(8):
                        w = 8 - i
                        lhs = bc(UT[:, :, i * 4 + j:i * 4 + j + 1], w)
                        rhs = Uv[:, :, i:8, j]
                        T(tmp[:, :, 0:w], lhs, rhs, MUL)
                        prun = XPZ[:, :, 12 + i * 8 + i: 12 + i * 8 + 8]
                        T(prun, prun, tmp[:, :, 0:w], SUB)

                P2 = XPZ[:, :, 12:76].rearrange("p f (i k) -> p f i k", i=8)
                for i in range(1, 8):
                    nc.scalar.copy(P2[:, :, i, 0:i], P2[:, :, 0:i, i])

                # ---- DMA out -------------------------------------------
                nc.sync.dma_start(out=ov[:, FS, 0:8], in_=XPZ[:, :, 0:8])
                nc.sync.dma_start(out=ov[:, FS, 8:72], in_=XPZ[:, :, 12:76])

    nc.compile()
    return nc


_prog_cache = {}


def kernel(x: np.ndarray, z: np.ndarray, P: np.ndarray,
           H: np.ndarray, R: np.ndarray) -> np.ndarray:
    from concourse.bass_utils import run_bass_kernel_spmd

    x = np.ascontiguousarray(x, dtype=np.float32).reshape(B, 8)
    z = np.ascontiguousarray(z, dtype=np.float32).reshape(B, 4)
    P = np.ascontiguousarray(P, dtype=np.float32).reshape(B, 64)
    W1 = _build_w1(np.asarray(H, np.float32), np.asarray(R, np.float32))
    ident = np.eye(128, dtype=np.float32)

    if "nc" not in _prog_cache:
        _prog_cache["nc"] = _build_program()
    nc = _prog_cache["nc"]

    in_maps = []
    for c in range(NCORES):
        s = slice(c * BC, (c + 1) * BC)
        in_maps.append({"xd": x[s], "zd": z[s], "Pd": P[s],
                        "w1d": W1, "idd": ident})
    res = run_bass_kernel_spmd(nc, in_maps, core_ids=list(range(NCORES)))
    out = np.concatenate([r["outd"].reshape(BC, 9, 8) for r in res.results],
                         axis=0)
    return out
